# revision 21
# baseline (speedup 1.0000x reference)
"""Trainium2 Bass kernel for nn_HFGA_54606214201918.

Computation (per batch element b, C=256 channels, L=4096 positions):
    xh  = (x[:, 0::2] - x[:, 1::2]) / sqrt(2)          # Haar high band  [C, L/2]
    q   = Wq @ x + bq                                  # [C, L]
    k   = Wk @ xh + bk                                 # [C, L/2]
    v   = Wv @ xh + bv                                 # [C, L/2]
    attn = softmax_over_keys((k^T q) / sqrt(C))        # [L/2, L]
    out = (v @ attn) * tanh(gate) + x

Sharding: data-parallel over batch B=8 across the 8 NeuronCores (one batch
element per core); weights are broadcast. No collectives needed.

Per-core design (from the 126us LAG-pipeline baseline; see git/backup
kernel_baseline.py for its docstring):
  - All heavy matmuls in fp8e4m3 DoubleRow (K=256/instruction, 216ns per
    N=512 MM = the real 157 TF/s fp8 roofline; cost is column-count only).
  - Scores land [keys, queries] in 2-bank PSUM tiles; one ACTIVATE(Exp)
    drains 1024 f32/partition and emits the fp8 E tile in DR rhs layout.
    1/sqrt(C) and a softmax-invariant -3 shift ride the ACT free affine.
  - bk is dropped entirely: k^T q picks up bk.q_l, constant over the key
    axis, which cancels exactly in the softmax.
  - bv is folded past the softmax: rows of normalized attn sum to 1, so
    v+bv contributes exactly bv*tanh(gate) per channel; it rides the final
    residual scalar_tensor_tensor add. This kills all rank-1 bias matmuls.
  - Software-pipelined rounds: round k computes scores+exp for l-tile k
    while the PE consumes the fully-buffered E of l-tile k-1 (Z ones-MM +
    v@E), so consume matmuls never wait on the exp stream. E tiles buffer
    one full l-tile ahead in SBUF (e-pool bufs=12).
  - Z accumulates in a persistent 2-row PSUM tile (subtile deps let rounds
    alternate rows with no false serialization). 1/Z is broadcast across
    partitions by gpsimd partition_broadcast (PE bcast-MM + copy removed).
  - PSUM = score 2x2 banks + yh 2 + qb 1 + z 1 = 8 banks exactly.
  - x streams in as 32 quarter-bank DMAs on the sync/gpsimd queues; the
    per-bank prologue (Haar sub, K/V projections) is fused into l-tile 0's
    pair loop gated with tc.tile_wait_until at real DMA arrival times.
  - Weights are host-pre-transposed to partition-contiguous layouts (the
    strided rearrange DMA cost ~6us on the scalar queue in the baseline).
  - 1/sqrt(2) and tanh(gate) are folded into Wk/Wv/bv on host.
"""
import sys

if '/opt/trn_rl_repo' not in sys.path:
    sys.path.insert(0, '/opt/trn_rl_repo')

import numpy as np

import concourse.bass as bass
import concourse.tile as tile
from concourse import bacc, mybir
from concourse import bass_utils
from concourse.alu_op_type import AluOpType

B, C, L = 8, 256, 4096
M = L // 2            # 2048 keys
P = 128               # partitions
CO = C // P           # 2 channel chunks
LB = 512              # l-tile (one PSUM bank of fp32)
NB = L // LB          # 8 l-tiles
MJ = M // P           # 16 key chunks
INV_SQRT2 = 0.7071067811865476

F32 = mybir.dt.float32
BF16 = mybir.dt.bfloat16
F8 = mybir.dt.float8e4
AF = mybir.ActivationFunctionType
DR = mybir.MatmulPerfMode.DoubleRow

EXP_SHIFT = -3.0      # softmax-invariant shift to fit E into fp8e4m3
EXP_SCALE = 1.0 / 16.0  # 1/sqrt(C)

_CACHE = {}


def _build():
    nc = bacc.Bacc("TRN2", target_bir_lowering=False, debug=False, num_devices=8)

    x_d = nc.dram_tensor("x", [C, L], F32, kind="ExternalInput").ap()
    wq_d = nc.dram_tensor("wqT", [P, CO * C], F8, kind="ExternalInput").ap()
    wk_d = nc.dram_tensor("wkT", [P, CO * C], F8, kind="ExternalInput").ap()
    wv_d = nc.dram_tensor("wvT", [P, CO * C], F8, kind="ExternalInput").ap()
    bq_d = nc.dram_tensor("bq", [P, CO], F32, kind="ExternalInput").ap()
    bvf_d = nc.dram_tensor("bvf", [P, CO], F32, kind="ExternalInput").ap()
    y_d = nc.dram_tensor("y", [C, L], F32, kind="ExternalOutput").ap()

    x3 = x_d.rearrange("(co ci) l -> ci co l", ci=P)      # [128, 2, 4096]
    y3 = y_d.rearrange("(co ci) l -> ci co l", ci=P)
    wq3 = wq_d.rearrange("p (cc o) -> p cc o", cc=CO)     # contiguous lhsT
    wk3 = wk_d.rearrange("p (cc o) -> p cc o", cc=CO)
    wv3 = wv_d.rearrange("p (cc o) -> p cc o", cc=CO)

    with tile.TileContext(nc) as tc:
        with tc.tile_pool(name="consts", bufs=1) as consts, \
             tc.tile_pool(name="big", bufs=1) as big, \
             tc.tile_pool(name="e", bufs=4) as e_pool, \
             tc.tile_pool(name="tmp", bufs=2) as tmp_pool, \
             tc.tile_pool(name="outp", bufs=2) as out_pool, \
             tc.tile_pool(name="ps", bufs=1, space="PSUM") as ps:

            # ---- constants (host pre-quantized to fp8, contiguous) ----
            bq_sb = consts.tile([P, CO], F32)
            bvf_sb = consts.tile([P, CO], F32)
            wq_r = consts.tile([P, CO, C], F8)            # DR lhsT/rhs
            wk_r = consts.tile([P, CO, C], F8)
            wv_r = consts.tile([P, CO, C], F8)

            ones_st = consts.tile([P, CO, 16], F32)
            nc.vector.memset(ones_st, 1.0)
            ones_dr = consts.tile([P, CO, 16], F8)        # DR ones lhsT for Z
            nc.vector.tensor_copy(ones_dr, ones_st)
            ones_row_f = consts.tile([1, P], F32)
            nc.vector.memset(ones_row_f, 1.0)
            ones_row_bf = consts.tile([1, P], BF16)       # K=1 bcast lhsT
            nc.vector.tensor_copy(ones_row_bf, ones_row_f)

            shift_col = consts.tile([P, 1], F32)          # exp bias column
            nc.vector.memset(shift_col, EXP_SHIFT)

            # preload the exp table set during the prologue
            dm = tmp_pool.tile([1, 16], F32, tag="rz", name="dummy_exp")
            nc.scalar.activation(dm, ones_st[0:1, 0, :], AF.Exp,
                                 bias=shift_col[0:1, :])

            # ---- big persistent tensors ----
            x_sb = big.tile([P, NB, CO, LB], F32)         # bank-major
            q_sb = big.tile([P, NB, CO, LB], F8)          # [o, l] bank-major
            xh_sb = big.tile([P, NB, CO, LB // 2], F8)    # [c, m] bank-major
            k_sb = big.tile([P, NB, CO, LB // 2], F8)     # [o, m] bank-major
            vt_sb = big.tile([P, MJ, C], F8)              # [m, o] chunks



            # ---- prologue DMAs: weights land first (contiguous, tiny);
            # x quarter-banks round-robin over sync/gpsimd/scalar so each
            # bank's 4 quarters split across all three queues ----
            nc.sync.dma_start(out=wk_r, in_=wk3)
            nc.gpsimd.dma_start(out=wv_r, in_=wv3)
            nc.scalar.dma_start(out=wq_r, in_=wq3)
            nc.scalar.dma_start(out=bq_sb, in_=bq_d)
            nc.scalar.dma_start(out=bvf_sb, in_=bvf_d)
            qrot = 0
            for idx in range(4 * NB):
                j, co, h = idx // 4, (idx % 4) // 2, idx % 2
                dst = x_sb[:, j, co, h * 256:(h + 1) * 256]
                srl = slice(j * LB + h * 256, j * LB + (h + 1) * 256)
                if j < 5:
                    eng = (nc.sync, nc.gpsimd)[idx % 2]
                else:   # late banks also ride the scalar queue (ACT is
                    eng = (nc.sync, nc.gpsimd, nc.scalar)[qrot % 3]
                    qrot += 1
                eng.dma_start(out=dst, in_=x3[:, co, srl])

            def x_bank(j):
                return x_sb[:, j]

            # ---- helpers ----
            xr_tiles = {}
            e_tiles = {}
            yhp_tiles = {}

            def xr_cast(k):
                xr = e_pool.tile([P, CO, LB], F8, tag="xr", bufs=3,
                                 name=f"xr{k}")
                nc.vector.tensor_copy(xr, x_bank(k))
                xr_tiles[k] = xr

            def q_proj_oc(k, oc):
                xr = xr_tiles[k] if oc == 0 else xr_tiles.pop(k)
                qp = ps.tile([P, LB], F32, tag="qb", bufs=1,
                             name=f"qp{k}_{oc}")
                nc.tensor.matmul(qp, wq_r[:, :, oc * P:(oc + 1) * P], xr,
                                 start=True, stop=True, perf_mode=DR)
                nc.vector.tensor_scalar_add(
                    q_sb[:, k, oc, :], qp, bq_sb[:, oc:oc + 1])

            def bank_prologue(j):
                # Haar high band (no 1/sqrt2: folded into wk/wv)
                pair = x_bank(j).rearrange("p c (m two) -> p c m two", two=2)
                nc.vector.tensor_sub(xh_sb[:, j], pair[:, :, :, 0],
                                     pair[:, :, :, 1])
                # K projection (bk dropped: softmax-invariant)
                kp = ps.tile([P, 2 * 256], F32, tag="qb", bufs=1,
                             name=f"kp{j}")
                for oc in range(CO):
                    nc.tensor.matmul(
                        kp[:, oc * 256:(oc + 1) * 256],
                        wk_r[:, :, oc * P:(oc + 1) * P],
                        xh_sb[:, j],
                        start=True, stop=True, perf_mode=DR)
                nc.scalar.copy(
                    k_sb[:, j], kp.rearrange("p (c m) -> p c m", c=CO))

                if j == 0:
                    xr_cast(0)
                    q_proj_oc(0, 0)
                    q_proj_oc(0, 1)
                if j == 1:
                    xr_cast(1)
                    q_proj_oc(1, 0)
                    q_proj_oc(1, 1)

                # V^T projection (bv folded into the final residual add)
                vp = ps.tile([P, 512], F32, tag="qb", bufs=1, name=f"vp{j}")
                for t in range(2):
                    nc.tensor.matmul(
                        vp[:, t * 256:(t + 1) * 256],
                        xh_sb[:, j, :, t * P:(t + 1) * P],
                        wv_r,
                        start=True, stop=True, perf_mode=DR)
                    nc.vector.tensor_copy(vt_sb[:, 2 * j + t, :],
                                          vp[:, t * 256:(t + 1) * 256])

            def scores(k, t):
                sp = ps.tile([P, 2 * LB], F32, tag="score", bufs=2,
                             name=f"sp{k}_{t}")
                for h in range(2):
                    nc.tensor.matmul(
                        sp[:, h * LB:(h + 1) * LB],
                        k_sb[:, t, :, h * P:(h + 1) * P],
                        q_sb[:, k],
                        start=True, stop=True, perf_mode=DR)
                return sp

            def exp_(k, t, sp):
                e2 = e_pool.tile([P, 2 * LB], F8, tag="e", bufs=12,
                                 name=f"e{k}_{t}")
                nc.scalar.activation(e2, sp, AF.Exp,
                                     bias=shift_col, scale=EXP_SCALE)
                e_tiles[(k, t)] = e2

            zp_tiles = {}

            def consume_z(k, t):
                if t == 0:
                    zp_tiles[k] = ps.tile([1, LB], F32, tag="z", bufs=1,
                                          name=f"zp{k}")
                e2v = e_tiles[(k, t)].rearrange("p (two l) -> p two l", two=2)
                nc.tensor.matmul(zp_tiles[k], ones_dr[:, :, 0:1],
                                 e2v, start=(t == 0), stop=(t == MJ // 2 - 1),
                                 perf_mode=DR)

            def consume_yh(k, t):
                if t == 0:
                    yhp_tiles[k] = [ps.tile([P, LB], F32, tag="yh", bufs=2,
                                            name=f"yh{k}_{i}")
                                    for i in range(CO)]
                e2v = e_tiles.pop((k, t)).rearrange(
                    "p (two l) -> p two l", two=2)
                for oc in range(CO):
                    nc.tensor.matmul(
                        yhp_tiles[k][oc],
                        vt_sb[:, 2 * t:2 * t + 2, oc * P:(oc + 1) * P], e2v,
                        start=(t == 0), stop=(t == MJ // 2 - 1), perf_mode=DR)

            rzb_tiles = {}

            def drain_prep(k):
                # zp(k) is complete; 1/Z as bf16 for the K=1 bcast matmul
                rz = tmp_pool.tile([1, LB], F32, tag="rz", name=f"rz{k}")
                nc.vector.reciprocal_approx_fast(out=rz, in_=zp_tiles.pop(k))
                rzb = tmp_pool.tile([1, LB], BF16, tag="rzb", name=f"rzb{k}")
                nc.vector.tensor_copy(rzb, rz)
                rzb_tiles[k] = rzb

            def drain_body(k, nchunk=1):
                # 1/Z broadcast via a K=1 matmul into the qb PSUM slot,
                # staged to SBUF on the scalar engine (ACT has round slack;
                # the DVE only runs the muls and fused residual adds).
                yhp = yhp_tiles.pop(k)
                bpp = ps.tile([P, LB], F32, tag="qb", bufs=1, name=f"bp{k}")
                nc.tensor.matmul(bpp, ones_row_bf, rzb_tiles.pop(k),
                                 start=True, stop=True)
                b_sb = tmp_pool.tile([P, LB], F32, tag="bsb", name=f"bsb{k}")
                nc.scalar.copy(b_sb, bpp)
                o_sb = out_pool.tile([P, CO, LB], F32, tag="o", name=f"o{k}")
                cw = LB // nchunk
                for ch in range(nchunk):
                    csl = slice(ch * cw, (ch + 1) * cw)
                    for oc in range(CO):
                        tm = tmp_pool.tile([P, cw], F32, tag="tm", bufs=4,
                                           name=f"t{k}_{oc}_{ch}")
                        nc.vector.tensor_mul(tm, yhp[oc][:, csl],
                                             b_sb[:, csl])
                        nc.vector.scalar_tensor_tensor(
                            out=o_sb[:, oc, csl], in0=tm,
                            scalar=bvf_sb[:, oc:oc + 1],
                            in1=x_bank(k)[:, oc, csl],
                            op0=AluOpType.add, op1=AluOpType.add)
                        dsl = slice(k * LB + ch * cw, k * LB + (ch + 1) * cw)
                        (nc.sync, nc.gpsimd)[(k + oc + ch) % 2].dma_start(
                            out=y3[:, oc, dsl], in_=o_sb[:, oc, csl])

            # ---- gated phase: prologue + scores/exp of l-tile 0 only, paced
            # by the real x-bank DMA arrival times; all tile-0 consumes are
            # deferred into round 1 (E tiles buffer in SBUF) ----
            NPAIR = MJ // 2
            for j in range(NPAIR):
                arr = 0.008 + 0.0021 * j
                with tc.tile_wait_until(arr):
                    bank_prologue(j)
                    exp_(0, j, scores(0, j))

            # ---- rounds k=1..7: scores/exp(k) + fully-buffered consume of
            # tile k-1; drain(k-2) rides the round start (its 1/Z prep ran
            # at the end of round k-1), yh consumes lag 2 steps so the
            # previous drain's muls release the yh PSUM slots in time ----
            for k in range(1, NB):
                kc = k - 1
                if kc >= 1:
                    drain_body(kc - 1)
                for t in range(NPAIR):
                    consume_z(kc, t)
                    sp = scores(k, t)
                    exp_(k, t, sp)
                    if t == 2 and k + 1 < NB:
                        xr_cast(k + 1)
                    if t == 4 and k + 1 < NB:
                        q_proj_oc(k + 1, 0)
                    if t == 6 and k + 1 < NB:
                        q_proj_oc(k + 1, 1)
                    if t >= 2:
                        consume_yh(kc, t - 2)
                for t in range(NPAIR - 2, NPAIR):
                    consume_yh(kc, t)
                drain_prep(kc)

            # ---- final round: consume tile 7 (Z first so the 1/Z prep and
            # broadcast overlap the yh matmuls), then a chunked drain ----
            drain_body(NB - 2)
            for t in range(NPAIR):
                consume_z(NB - 1, t)
            drain_prep(NB - 1)
            for t in range(NPAIR):
                consume_yh(NB - 1, t)
            drain_body(NB - 1, nchunk=2)

    nc.compile()
    return nc


def _get_nc():
    if "nc" not in _CACHE:
        _CACHE["nc"] = _build()
    return _CACHE["nc"]


def _prep_inputs(x, Wq, bq, Wk, bk, Wv, bv, attn_gate):
    x = np.asarray(x, dtype=np.float32)
    Wq = np.asarray(Wq, dtype=np.float32)
    Wk = np.asarray(Wk, dtype=np.float32)
    Wv = np.asarray(Wv, dtype=np.float32)
    bq = np.asarray(bq, dtype=np.float32)
    bv = np.asarray(bv, dtype=np.float32)
    gate = float(np.tanh(np.asarray(attn_gate, dtype=np.float64))[0])

    import ml_dtypes
    f8np = ml_dtypes.float8_e4m3
    # lhsT layouts [c_in, c_out] pre-transposed to the SBUF-contiguous
    # [ci][cc, o] layout so each weight DMA is one burst per partition.
    # Haar 1/sqrt(2) folds into wk/wv; tanh(gate) into wv and bv.
    # bk is dropped (softmax-invariant); bv applies post-softmax.
    def prep_w(w):
        wT = np.ascontiguousarray(w.T).astype(f8np)       # [c_in, c_out]
        return np.ascontiguousarray(
            wT.reshape(CO, P, C).transpose(1, 0, 2).reshape(P, CO * C))

    wqT = prep_w(Wq)
    wkT = prep_w(Wk * np.float32(INV_SQRT2))
    wvT = prep_w(Wv * np.float32(INV_SQRT2 * gate))
    bq2 = np.ascontiguousarray(bq.reshape(CO, P).T)       # [P, CO]
    bvf2 = np.ascontiguousarray((bv * np.float32(gate)).reshape(CO, P).T)
    return [{
        "x": np.ascontiguousarray(x[b]),
        "wqT": wqT, "wkT": wkT, "wvT": wvT,
        "bq": bq2, "bvf": bvf2,
    } for b in range(B)]


def kernel(x, Wq, bq, Wk, bk, Wv, bv, attn_gate, _run_kwargs=None):
    in_maps = _prep_inputs(x, Wq, bq, Wk, bk, Wv, bv, attn_gate)
    nc = _get_nc()
    res = bass_utils.run_bass_kernel_spmd(
        nc, in_maps, core_ids=list(range(B)), **(_run_kwargs or {}))
    out = np.stack([res.results[b]["y"] for b in range(B)]).astype(np.float32)
    if _run_kwargs:
        kernel.last_results = res
    return out
